# revision 7
# baseline (speedup 1.0000x reference)
"""Multi-head attention (B=4, S=2048, D=1024, H=16) on 8 Trainium2 cores.

Sharding: core c = (batch b = c//2, head-group g = c%2). Each core computes
8 heads' attention for one batch element plus the partial output projection
for its head-group's rows of Wo; the host sums the two partials per batch
and adds the bias.

Per-core pipeline (fp8 DoubleRow for projections/ctx/den, bf16 scores and
output projection, softmax denominator via an all-ones fp8 weight matmul
that lands replicated across all psum partitions so normalization is
partition-aligned):

  xt        [D, S] fp8      (host: transpose + cast)
  q/k/v     = DR-matmul over K-chunk pairs; q,k -> bf16 SBUF, v -> fp8 SBUF
  scores    s2 = k_h.T @ q_h per 128-key block, bf16, two heads row-packed
            (q weights pre-scaled by log2(e): s2 = raw_score * log2e)
  exp       split by key-block-pair between ScalarE (ACT Exp -> fp8, bias
            delta) and DVE (Schraudolph: i8 = max(s2,-B8)+B8, int8 bitcast
            as fp8e4; same global scale 2^((B8-55.536)/8), cancels in the
            softmax ratio)
  ctx       DR-matmul over key-block pairs, M=64 per head -> cacc[0:64]/
            cacc[64:128] (head A/B)
  den       DR-matmul lhsT=ones[128,2,128] -> dacc[:, headhalf] replicated
            across all 128 partitions
  norm      rec = 1/dacc (DVE); ctxT = cacc * rec (aligned halves)
  out       ctxT.T @ Wo -> fp32 partial to DRAM
"""

import numpy as np

B, S, D = 4, 2048, 1024
H, HD = 16, 64
NCORES = 8
G = D // 2          # head-group width per core (8 heads x 64)
P = 128
DC = D // P         # d_in chunks
GC = G // P         # head-pair chunks
SB = S // P         # key blocks
W = 512             # q-chunk width
NW = S // W

LOG2E = float(np.log2(np.e))
B8 = 43.0                                  # Schraudolph int8 bias (top-safety: s2max=74.9)
B8_NEUTRAL = (7 - 0.0573) * 8.0            # 55.536
DELTA = (B8 - B8_NEUTRAL) * float(np.log(2.0)) / 8.0   # ACT bias matching DVE scale
DVE_KBP = (1, 4, 6)                        # key-block pairs whose exp runs on DVE

_BUILD_CACHE = {}


def build_mha():
    key = "mha"
    if key in _BUILD_CACHE:
        return _BUILD_CACHE[key]

    import concourse.bacc as bacc
    import concourse.mybir as mybir
    import concourse.tile as tile
    from contextlib import ExitStack, nullcontext

    FP32 = mybir.dt.float32
    BF16 = mybir.dt.bfloat16
    FP8 = mybir.dt.float8e4
    I8 = mybir.dt.int8
    DR = mybir.MatmulPerfMode.DoubleRow
    EXP = mybir.ActivationFunctionType.Exp
    MUL = mybir.AluOpType.mult
    MAX = mybir.AluOpType.max
    ADD = mybir.AluOpType.add

    nc = bacc.Bacc("TRN2", target_bir_lowering=False, debug=False)
    xt_d = nc.declare_dram_parameter("xt", [D, S], FP8, isOutput=False)
    wq_d = nc.declare_dram_parameter("wq", [D, G], FP8, isOutput=False)
    wk_d = nc.declare_dram_parameter("wk", [D, G], FP8, isOutput=False)
    wv_d = nc.declare_dram_parameter("wv", [D, G], FP8, isOutput=False)
    wo_d = nc.declare_dram_parameter("wo", [G, D], mybir.dt.bfloat16, isOutput=False)
    out_d = nc.declare_dram_parameter("out", [S, D], FP32, isOutput=True)

    with tile.TileContext(nc) as tc, ExitStack() as ctx:
        const = ctx.enter_context(tc.tile_pool(name="const", bufs=1))
        wpool = ctx.enter_context(tc.tile_pool(name="wpool", bufs=1))
        big = ctx.enter_context(tc.tile_pool(name="big", bufs=1))
        ppool = ctx.enter_context(tc.tile_pool(name="ppool", bufs=3))
        norm = ctx.enter_context(tc.tile_pool(name="norm", bufs=2))
        cbpool = ctx.enter_context(tc.tile_pool(name="cbpool", bufs=2))
        outp = ctx.enter_context(tc.tile_pool(name="outp", bufs=4))
        pscore = ctx.enter_context(tc.tile_pool(name="pscore", bufs=2, space="PSUM"))
        pcacc = ctx.enter_context(tc.tile_pool(name="pcacc", bufs=2, space="PSUM"))
        pdacc = ctx.enter_context(tc.tile_pool(name="pdacc", bufs=1, space="PSUM"))

        ones_f8 = const.tile([P, 2, P], FP8)
        nc.gpsimd.memset(ones_f8[:], 1.0)
        dbias = const.tile([P, 1], FP32)
        nc.gpsimd.memset(dbias[:], DELTA)

        wq_sb = wpool.tile([P, DC, G], FP8)
        wk_sb = wpool.tile([P, DC, G], FP8)
        wv_sb = wpool.tile([P, DC, G], FP8)
        wo_sb = wpool.tile([P, GC, D], BF16)
        xt = big.tile([P, DC, S], FP8)

        for dc in range(DC):
            nc.sync.dma_start(xt[:, dc, :], xt_d[dc * P:(dc + 1) * P, :])
        for w_d, w_sb, nch in (
            (wq_d, wq_sb, DC),
            (wk_d, wk_sb, DC),
            (wv_d, wv_sb, DC),
            (wo_d, wo_sb, GC),
        ):
            for c in range(nch):
                nc.sync.dma_start(w_sb[:, c, :], w_d[c * P:(c + 1) * P, :])

        qt = big.tile([P, GC, S], BF16)
        kt = big.tile([P, GC, S], BF16)
        vp = big.tile([P, SB, G], FP8)
        ctxT = big.tile([P, GC, S], BF16)

        def emit_proj_qk(g):
            for w_sb, dst in ((wq_sb, qt), (wk_sb, kt)):
                for sc in range(S // 512):
                    ps = pscore.tile([P, 512], FP32, tag="pscore", name="ps")
                    for cp in range(DC // 2):
                        nc.tensor.matmul(
                            ps[:],
                            lhsT=w_sb[:, 2 * cp:2 * cp + 2, g * P:(g + 1) * P],
                            rhs=xt[:, 2 * cp:2 * cp + 2, sc * 512:(sc + 1) * 512],
                            start=(cp == 0),
                            stop=(cp == DC // 2 - 1),
                            perf_mode=DR,
                        )
                    nc.vector.tensor_copy(dst[:, g, sc * 512:(sc + 1) * 512], ps[:])

        def emit_proj_v(sb):
            ps = pscore.tile([P, G], FP32, tag="pscore", name="ps")
            for cp in range(DC // 2):
                nc.tensor.matmul(
                    ps[:],
                    lhsT=xt[:, 2 * cp:2 * cp + 2, sb * P:(sb + 1) * P],
                    rhs=wv_sb[:, 2 * cp:2 * cp + 2, :],
                    start=(cp == 0),
                    stop=(cp == DC // 2 - 1),
                    perf_mode=DR,
                )
            nc.vector.tensor_copy(vp[:, sb, :], ps[:])

        emit_proj_qk(0)
        for sb in range(min(4, SB)):
            emit_proj_v(sb)

        scale = float(np.log(2.0) / 8.0)
        NJ = SB // 2

        for qw in range(NW):
            q0 = qw * W
            for p in range(GC):
                if qw == 0 and p > 0:
                    with tc.high_priority(offset=-1_000_000):
                        emit_proj_qk(p)
                hA, hB = 2 * p, 2 * p + 1
                # DoubleRow matmuls require dst partition base 0, so each
                # head accumulates in its own bank at partitions 0:64.
                cacc_a = pcacc.tile([P, 512], FP32, tag="pcacc", name="cacc_a")
                cacc_b = pcacc.tile([P, 512], FP32, tag="pcacc", name="cacc_b")
                dacc = pdacc.tile([P, 1024], FP32, tag="pdacc", name="dacc")

                def emit_scores_exp(j):
                    pt = ppool.tile([P, 2, 1024], FP8, tag="ppool", name="pt")
                    for parity in (0, 1):
                        kb = 2 * j + parity
                        s = pscore.tile([P, 1024], FP32, tag="pscore", name="s")
                        nc.tensor.matmul(
                            s[:, 0:512],
                            lhsT=kt[0:64, p, kb * P:(kb + 1) * P],
                            rhs=qt[0:64, p, q0:q0 + 512],
                            start=True, stop=True,
                        )
                        nc.tensor.matmul(
                            s[:, 512:1024],
                            lhsT=kt[64:128, p, kb * P:(kb + 1) * P],
                            rhs=qt[64:128, p, q0:q0 + 512],
                            start=True, stop=True,
                        )
                        if j in DVE_KBP:
                            nc.vector.tensor_scalar(
                                pt[:, parity, :].bitcast(I8), s[:],
                                -B8, B8, MAX, ADD,
                            )
                        else:
                            nc.scalar.activation(
                                pt[:, parity, :], s[:], EXP,
                                bias=dbias[:], scale=scale,
                            )
                    return pt

                def emit_ctx_den(j, pt):
                    first, last = j == 0, j == NJ - 1
                    nc.tensor.matmul(
                        cacc_a[0:64, :],
                        lhsT=vp[:, 2 * j:2 * j + 2, hA * HD:(hA + 1) * HD],
                        rhs=pt[:, :, 0:512],
                        start=first, stop=last,
                        perf_mode=DR, skip_group_check=True,
                    )
                    nc.tensor.matmul(
                        cacc_b[0:64, :],
                        lhsT=vp[:, 2 * j:2 * j + 2, hB * HD:(hB + 1) * HD],
                        rhs=pt[:, :, 512:1024],
                        start=first, stop=last,
                        perf_mode=DR, skip_group_check=True,
                    )
                    nc.tensor.matmul(
                        dacc[:, 0:512],
                        lhsT=ones_f8[:],
                        rhs=pt[:, :, 0:512],
                        start=first, stop=last,
                        perf_mode=DR, skip_group_check=True,
                    )
                    nc.tensor.matmul(
                        dacc[:, 512:1024],
                        lhsT=ones_f8[:],
                        rhs=pt[:, :, 512:1024],
                        start=first, stop=last,
                        perf_mode=DR, skip_group_check=True,
                    )

                prev = None
                for j in range(NJ):
                    cur = emit_scores_exp(j)
                    if prev is not None:
                        emit_ctx_den(j - 1, prev)
                    if qw == 0 and p == 0:
                        for sb in (4 + 2 * j, 5 + 2 * j):
                            if sb < SB:
                                emit_proj_v(sb)
                    prev = cur
                emit_ctx_den(NJ - 1, prev)

                rec = norm.tile([P, 1024], FP32, tag="rec")
                nc.vector.reciprocal_approx_fast(rec[0:64, :], dacc[0:64, :])
                nc.vector.tensor_tensor(
                    ctxT[0:64, p, q0:q0 + 512],
                    cacc_a[0:64, :], rec[0:64, 0:512], MUL,
                )
                # head B lands at partitions 0:64 (DR dst constraint); DVE
                # lanes are partition-locked, so normalize there and let a
                # SBUF->SBUF DMA move it to ctxT's upper half.
                cbn = cbpool.tile([P, 512], BF16, tag="cbn")
                nc.vector.tensor_tensor(
                    cbn[0:64, :], cacc_b[0:64, :], rec[0:64, 512:1024], MUL,
                )
                nc.sync.dma_start(ctxT[64:128, p, q0:q0 + 512], cbn[0:64, :])

            last_qw = qw == NW - 1
            prio = nullcontext() if last_qw else tc.high_priority(offset=-1_000_000)
            with prio:
                for sb in range(W // P):
                    row = q0 + sb * P
                    for nck in range(D // 512):
                        po = pscore.tile([P, 512], FP32, tag="pscore", name="po")
                        for g in range(GC):
                            nc.tensor.matmul(
                                po[:],
                                lhsT=ctxT[:, g, row:row + P],
                                rhs=wo_sb[:, g, nck * 512:(nck + 1) * 512],
                                start=(g == 0),
                                stop=(g == GC - 1),
                            )
                        ob = outp.tile([P, 512], FP32, tag="ob")
                        nc.vector.tensor_copy(ob[:], po[:])
                        nc.sync.dma_start(
                            out_d[row:row + P, nck * 512:(nck + 1) * 512], ob[:]
                        )

    nc.compile()
    _BUILD_CACHE[key] = nc
    return nc


def make_shards(x, Wq, Wk, Wv, Wo):
    """Split full inputs into 8 per-core input maps (host-side layout prep)."""
    import ml_dtypes
    F8 = ml_dtypes.float8_e4m3
    BF = ml_dtypes.bfloat16
    x = np.asarray(x, dtype=np.float32)
    xt = np.ascontiguousarray(x.transpose(0, 2, 1)).astype(F8)   # [B, D, S]
    Wqf = (np.asarray(Wq, dtype=np.float32) * LOG2E).astype(F8)
    Wkf = np.asarray(Wk, dtype=np.float32).astype(F8)
    Wvf = np.asarray(Wv, dtype=np.float32).astype(F8)
    Wob = np.asarray(Wo, dtype=np.float32).astype(BF)
    shards = []
    for c in range(NCORES):
        b, g = divmod(c, 2)
        cs = slice(g * G, (g + 1) * G)
        shards.append({
            "xt": xt[b],
            "wq": np.ascontiguousarray(Wqf[:, cs]),
            "wk": np.ascontiguousarray(Wkf[:, cs]),
            "wv": np.ascontiguousarray(Wvf[:, cs]),
            "wo": np.ascontiguousarray(Wob[cs, :]),
        })
    return shards


def combine(results, bo):
    bo = np.asarray(bo, dtype=np.float32)
    outs = [results[c]["out"] for c in range(NCORES)]
    return np.stack([outs[2 * b] + outs[2 * b + 1] for b in range(B)]) + bo


def run_shards(shards, trace=False, **kw):
    from concourse.bass_utils import run_bass_kernel_spmd
    nc = build_mha()
    return run_bass_kernel_spmd(nc, shards, list(range(NCORES)), trace=trace, **kw)


def kernel(x, Wq, Wk, Wv, Wo, bo):
    res = run_shards(make_shards(x, Wq, Wk, Wv, Wo))
    return combine(res.results, bo)
